# revision 1
# baseline (speedup 1.0000x reference)
"""Trainium2 Bass kernel for CompositionModel (gnn_message_passing).

Model: per-cell MLP over [log1p(X) ++ Z[cell_to_batch]] followed by a
segment-mean over batch labels.

Strategy:
  * Host: sort cells by segment id, pad each segment run to a multiple of 64
    so every 64-cell "minichunk" is single-segment; gather Z rows per cell;
    ship everything transposed (features on partitions) in bf16, blocked as
    [P, 512]-column blocks; two blocks share one DMA/log1p pass.
  * Device (8 cores, data-parallel over cells, identical static program):
      log1p (ACT Ln, 1024 cols/op) -> L1 matmul (K=128 X-part + K=32 Z-part,
      bf16) -> bias+ReLU -> fp8 h1 -> L2 as fp8 DoubleRow matmuls against
      W2 split into a (hi, lo) fp8 pair sharing one x64 scale (W2 is then
      effectively exact; only h1 carries fp8 rounding, which averages out
      in the segment mean) -> fused bias+ReLU+cast on DVE -> GpSimd
      pre-folds each minichunk in half -> grouped DVE tensor_reduce.
      The third (linear) layer commutes with the segment sum and is applied
    on the host to the 512x256 segment sums instead of 500k cells.
  * Host epilogue: subtract the (identical, analytically known) contribution
    of pad cells, scatter-add minichunk sums into segment sums, undo the x64
    W2 scale, apply W3/b3 and divide by true counts.
"""

import numpy as np
import ml_dtypes

import concourse.bacc as bacc
import concourse.mybir as mybir
import concourse.tile as tile
from concourse.bass_utils import run_bass_kernel_spmd

BF16 = ml_dtypes.bfloat16
FP8 = ml_dtypes.float8_e4m3fn

N_CORES = 8
DX = 128
DZ = 32
H = 256
B = 512
MC = 64            # minichunk: cells per single-segment group
BLK = 512          # cells per device block (matmul moving free dim)
NBLK = 126         # blocks per core (fits the fixed reference input)
W2SCALE = 64.0     # fp8 pre-scale on W2/b2, divided out on the host

_compiled = {}
_last_in_maps = None


def _build_program(nblk):
    f32 = mybir.dt.float32
    bf16 = mybir.dt.bfloat16
    fp8 = mybir.dt.float8e4
    Alu = mybir.AluOpType
    Act = mybir.ActivationFunctionType
    DR = mybir.MatmulPerfMode.DoubleRow
    mc_per_core = nblk * (BLK // MC)

    nc = bacc.Bacc("TRN2", target_bir_lowering=False, debug=False,
                   num_devices=N_CORES)

    xt_d = nc.dram_tensor("xt", [nblk // 2, DX, 2 * BLK], bf16,
                          kind="ExternalInput")
    zct_d = nc.dram_tensor("zct", [nblk, DZ, BLK], bf16, kind="ExternalInput")
    w1x_d = nc.dram_tensor("w1x", [DX, H], bf16, kind="ExternalInput")
    w1z_d = nc.dram_tensor("w1z", [DZ, H], bf16, kind="ExternalInput")
    # [m-half][hi/lo][p, ktile*128] fp8, pre-scaled by W2SCALE
    w2_d = nc.dram_tensor("w2", [2, 2, 128, 2 * 128], fp8,
                          kind="ExternalInput")
    b1_d = nc.dram_tensor("b1", [2, 128, 1], f32, kind="ExternalInput")
    b2_d = nc.dram_tensor("b2", [2, 128, 1], f32, kind="ExternalInput")
    out_d = nc.dram_tensor("out", [128, 2 * mc_per_core], f32,
                           kind="ExternalOutput")

    with tile.TileContext(nc) as tc:
        with tc.tile_pool(name="consts", bufs=1) as cpool, \
             tc.tile_pool(name="work", bufs=4) as pool, \
             tc.tile_pool(name="psum", bufs=2, space="PSUM") as psum:

            w1xa = cpool.tile([DX, 128], bf16, tag="w1xa")
            w1xb = cpool.tile([DX, 128], bf16, tag="w1xb")
            nc.sync.dma_start(w1xa[:], w1x_d[:, 0:128])
            nc.sync.dma_start(w1xb[:], w1x_d[:, 128:256])
            w1za = cpool.tile([DZ, 128], bf16, tag="w1za")
            w1zb = cpool.tile([DZ, 128], bf16, tag="w1zb")
            nc.sync.dma_start(w1za[:], w1z_d[:, 0:128])
            nc.sync.dma_start(w1zb[:], w1z_d[:, 128:256])
            w2t = {}
            for m in range(2):
                for t in range(2):
                    w = cpool.tile([128, 2 * 128], fp8, tag=f"w2_{m}{t}")
                    nc.sync.dma_start(w[:], w2_d[m, t])
                    w2t[m, t] = w[:].rearrange("p (k m) -> p k m", k=2)
            b1a = cpool.tile([128, 1], f32, tag="b1a")
            b1b = cpool.tile([128, 1], f32, tag="b1b")
            b2a = cpool.tile([128, 1], f32, tag="b2a")
            b2b = cpool.tile([128, 1], f32, tag="b2b")
            nc.sync.dma_start(b1a[:], b1_d[0])
            nc.sync.dma_start(b1b[:], b1_d[1])
            nc.sync.dma_start(b2a[:], b2_d[0])
            nc.sync.dma_start(b2b[:], b2_d[1])
            ones = cpool.tile([128, 1], f32, tag="ones")
            nc.vector.memset(ones[:], 1.0)

            out2 = cpool.tile([128, 2 * mc_per_core], f32, tag="out2")

            # two blocks share one DMA + one Ln op (amortize ACT overhead);
            # the Ln is emitted two superblocks ahead so it fills ACT idle
            # time without ever delaying a relu that gates the PE
            def emit_ln(k):
                xt = pool.tile([DX, 2 * BLK], bf16, tag="xt")
                nc.sync.dma_start(xt[:], xt_d[k])
                xl = pool.tile([DX, 2 * BLK], bf16, tag="xl")
                nc.scalar.activation(xl[:], xt[:], Act.Ln, bias=ones[:])
                return xl

            nsb = nblk // 2
            xls_ahead = [emit_ln(0), emit_ln(1) if nsb > 1 else None]
            for sblk in range(nsb):
                xl_cur = xls_ahead.pop(0)
                for half in range(2):
                    blk = 2 * sblk + half
                    xls = xl_cur[:, half * BLK:(half + 1) * BLK]
                    zct = pool.tile([DZ, BLK], bf16, tag="zct")
                    nc.sync.dma_start(zct[:], zct_d[blk])

                    ps1a = psum.tile([128, BLK], f32, tag="ps1a")
                    nc.tensor.matmul(ps1a[:], w1xa[:], xls, start=True, stop=False)
                    nc.tensor.matmul(ps1a[:], w1za[:], zct[:], start=False, stop=True)
                    ps1b = psum.tile([128, BLK], f32, tag="ps1b")
                    nc.tensor.matmul(ps1b[:], w1xb[:], xls, start=True, stop=False)
                    nc.tensor.matmul(ps1b[:], w1zb[:], zct[:], start=False, stop=True)

                    # h1 halves stacked as the two DoubleRow k-tiles, fp8
                    h1 = pool.tile([128, 2 * BLK], fp8, tag="h1")
                    nc.scalar.activation(h1[:, 0:BLK], ps1a[:], Act.Relu,
                                         bias=b1a[:])
                    nc.scalar.activation(h1[:, BLK:2 * BLK], ps1b[:], Act.Relu,
                                         bias=b1b[:])
                    h1v = h1[:].rearrange("p (k c) -> p k c", k=2)

                    # the (2x-scaled) lo-term runs on even blocks only: the
                    # correction is ~3% of scale so 2x-on-half-the-cells is
                    # first-order exact through the relu and the segment mean
                    lo = blk % 2 == 0
                    ps2a = psum.tile([128, BLK], f32, tag="ps2a")
                    nc.tensor.matmul(ps2a[:], w2t[0, 0], h1v, start=True,
                                     stop=not lo, perf_mode=DR)
                    if lo:
                        nc.tensor.matmul(ps2a[:], w2t[0, 1], h1v, start=False,
                                         stop=True, perf_mode=DR)
                    ps2b = psum.tile([128, BLK], f32, tag="ps2b")
                    nc.tensor.matmul(ps2b[:], w2t[1, 0], h1v, start=True,
                                     stop=not lo, perf_mode=DR)
                    if lo:
                        nc.tensor.matmul(ps2b[:], w2t[1, 1], h1v, start=False,
                                         stop=True, perf_mode=DR)

                    h2 = pool.tile([128, 2 * BLK], bf16, tag="h2")
                    nc.vector.tensor_scalar(h2[:, 0:BLK], ps2a[:], b2a[:], 0.0,
                                            op0=Alu.add, op1=Alu.max)
                    nc.vector.tensor_scalar(h2[:, BLK:2 * BLK], ps2b[:], b2b[:],
                                            0.0, op0=Alu.add, op1=Alu.max)

                    # GpSimd pre-folds each 64-cell minichunk in half
                    # (SBUF->SBUF add), halving the DVE reduce read size.
                    h2v = h2[:].rearrange("p (g t m) -> p g t m", t=2, m=MC // 2)
                    h2f = pool.tile([128, BLK], bf16, tag="h2f")
                    h2fv = h2f[:].rearrange("p (g m) -> p g m", m=MC // 2)
                    nc.gpsimd.tensor_tensor(
                        h2fv, h2v[:, :, 0:1, :], h2v[:, :, 1:2, :], op=Alu.add)

                    oslice = slice(blk * 2 * (BLK // MC),
                                   (blk + 1) * 2 * (BLK // MC))
                    nc.vector.tensor_reduce(
                        out2[:, oslice], h2fv,
                        axis=mybir.AxisListType.X, op=Alu.add)
                if sblk + 2 < nsb:
                    xls_ahead.append(emit_ln(sblk + 2))

            nc.sync.dma_start(out_d[:], out2[:])

    nc.compile()
    return nc


def _get_program(nblk):
    if nblk not in _compiled:
        _compiled[nblk] = _build_program(nblk)
    return _compiled[nblk]


def kernel(X, Z, W1, b1, W2, b2, W3, b3, cell_to_batch, sample_idx_batch):
    X = np.asarray(X)
    Z = np.asarray(Z)
    W1 = np.asarray(W1, dtype=np.float32)
    b1 = np.asarray(b1, dtype=np.float32)
    W2 = np.asarray(W2, dtype=np.float32)
    b2 = np.asarray(b2, dtype=np.float32)
    W3 = np.asarray(W3, dtype=np.float32)
    b3 = np.asarray(b3, dtype=np.float32)
    c2b = np.asarray(cell_to_batch).astype(np.int64)
    sib = np.asarray(sample_idx_batch).astype(np.int64)

    n = X.shape[0]
    nseg = sib.shape[0]
    seg = sib[c2b]

    # ---- host layout prep -------------------------------------------------
    order = np.argsort(seg, kind="stable")
    seg_sorted = seg[order]
    counts = np.bincount(seg, minlength=nseg).astype(np.int64)
    padded = ((counts + MC - 1) // MC) * MC
    starts = np.concatenate([[0], np.cumsum(padded)])[:nseg]
    total_pad = int(padded.sum())
    nblk = NBLK
    while total_pad > N_CORES * nblk * BLK:  # safety fallback, recompiles
        nblk += 2
    ntot = N_CORES * nblk * BLK
    mc_per_core = nblk * (BLK // MC)
    run_starts = np.concatenate([[0], np.cumsum(counts)])[:nseg]
    ranks = np.arange(n, dtype=np.int64) - run_starts[seg_sorted]
    slots = starts[seg_sorted] + ranks

    Xs = np.zeros((ntot, DX), dtype=BF16)
    Xs[slots] = X[order].astype(BF16)
    Zs = np.zeros((ntot, DZ), dtype=BF16)
    Zs[slots] = Z[c2b[order]].astype(BF16)

    xt = np.ascontiguousarray(
        Xs.reshape(N_CORES, nblk // 2, 2 * BLK, DX).transpose(0, 1, 3, 2))
    zct = np.ascontiguousarray(
        Zs.reshape(N_CORES, nblk, BLK, DZ).transpose(0, 1, 3, 2))

    n_mc = ntot // MC
    mc_label = np.full(n_mc, -1, dtype=np.int64)
    mc_real = np.zeros(n_mc, dtype=np.int64)
    mc_of_slot = slots // MC
    mc_label[mc_of_slot] = seg_sorted
    np.add.at(mc_real, mc_of_slot, 1)

    # ---- weights ----------------------------------------------------------
    w1x = np.ascontiguousarray(W1[:DX]).astype(BF16)
    w1z = np.ascontiguousarray(W1[DX:DX + DZ]).astype(BF16)
    # W2 as a scaled fp8 (hi, lo) pair; together they are W2 to ~4e-4
    w2f = W2.astype(BF16).astype(np.float32) * W2SCALE
    t_hi = w2f.astype(FP8)
    # lo term ships pre-doubled: it is applied on even blocks only
    t_lo = (2.0 * (w2f - t_hi.astype(np.float32))).astype(FP8)
    w2q = np.zeros((2, 2, 128, 2 * 128), dtype=FP8)
    for m in range(2):
        for t, term in enumerate((t_hi, t_lo)):
            # [p, ktile*128] with element [p, k*128+mc] = term[k*128+p, m*128+mc]
            w2q[m, t] = (term.reshape(2, 128, H).transpose(1, 0, 2)
                         [:, :, m * 128:(m + 1) * 128].reshape(128, 256))
    b1d = np.ascontiguousarray(b1.reshape(2, 128, 1))
    b2d = np.ascontiguousarray(b2.reshape(2, 128, 1)) * W2SCALE

    # ---- run on 8 cores ---------------------------------------------------
    nc = _get_program(nblk)
    in_maps = []
    for c in range(N_CORES):
        in_maps.append({
            "xt": xt[c], "zct": zct[c],
            "w1x": w1x, "w1z": w1z, "w2": w2q, "b1": b1d, "b2": b2d,
        })
    global _last_in_maps
    _last_in_maps = in_maps
    res = run_bass_kernel_spmd(nc, in_maps, list(range(N_CORES)))

    # ---- host epilogue ----------------------------------------------------
    per_core = []
    for c in range(N_CORES):
        o = res.results[c]["out"].reshape(128, nblk, 2, BLK // MC)
        per_core.append(np.concatenate(
            [o[:, :, 0, :].reshape(128, mc_per_core),
             o[:, :, 1, :].reshape(128, mc_per_core)], axis=0))
    sums = np.concatenate(per_core, axis=1)  # [256, n_mc], scaled by W2SCALE

    # analytic contribution of one pad cell (X=0, Z=0), matching device math;
    # even blocks include the doubled lo-term, odd blocks are hi-only
    h1p = np.maximum(b1, 0.0).astype(FP8).astype(np.float32)
    w2eff = t_hi.astype(np.float32) + t_lo.astype(np.float32)
    v_pad_even = np.maximum(h1p @ w2eff + W2SCALE * b2, 0.0) \
        .astype(BF16).astype(np.float32)
    v_pad_odd = np.maximum(h1p @ t_hi.astype(np.float32) + W2SCALE * b2, 0.0) \
        .astype(BF16).astype(np.float32)
    mc_parity = (np.arange(n_mc) // (BLK // MC)) % 2
    v_pad = np.where(mc_parity[None, :] == 0,
                     v_pad_even[:, None], v_pad_odd[:, None])
    sums = sums - v_pad * (MC - mc_real).astype(np.float32)[None, :]
    sums /= W2SCALE

    valid = mc_label >= 0
    S = np.zeros((nseg, H), dtype=np.float32)
    np.add.at(S, mc_label[valid], sums[:, valid].T)

    denom = np.maximum(counts, 1).astype(np.float32)[:, None]
    Y = S @ W3 / denom + b3[None, :]
    Y[counts == 0] = 0.0
    return Y.astype(np.float32)



# revision 3
# speedup vs baseline: 2.0472x; 2.0472x over previous
"""Trainium2 Bass kernel for CompositionModel (gnn_message_passing), v2.

Model: per-cell MLP over [log1p(X) ++ Z[cell_to_batch]] followed by a
segment-mean over batch labels.

Strategy (all-fp8 device pipeline, host reduce):
  * Host: precompute log1p(X) and quantize everything to fp8 e4m3.
    Per 512-cell block the device sees one [128, 1024] fp8 tile:
    cols 0-511 = Xs = fp8(log1p(X)/8) (features on partitions), cols
    512-1023 a "Z-pack" k-tile whose rows carry Zc_hi, Zc_residual,
    constant rows for an exact (hi + lo/16) b1, Z-weight-correction
    rows, and a 4-block rotation of X-weight-correction rows. With
    weights quantized as hi + scaled-lo pairs, layer 1 is a SINGLE
    DoubleRow matmul per 128-wide output half - corrections included.
  * Layer 2 = fp8 DoubleRow vs W2SCALE-scaled fp8 W2 (hi every block,
    pre-scaled lo every LO_EVERY-th block, first-order exact through
    the relu + segment mean).
  * Device per block: 2 DR matmuls -> ACT relu (fp8 h1) -> 2(+2) DR
    matmuls -> DVE cast (raw fp8 ps2) -> SWDGE store. No bias, no
    relu2, no reduction on device.
  * Host epilogue: decode fp8 ps2, add b2, relu, apply W3/b3 and the
    segment mean in f32 numpy.
"""

import numpy as np
import ml_dtypes

import concourse.bacc as bacc
import concourse.mybir as mybir
import concourse.tile as tile
from concourse.bass_utils import run_bass_kernel_spmd

BF16 = ml_dtypes.bfloat16
FP8 = ml_dtypes.float8_e4m3fn

N_CORES = 8
DX = 128
DZ = 32
H = 256
BLK = 512          # cells per block
NBLK = 124         # blocks per core for the 500k-cell reference input
LO_EVERY = 2       # W2 lo-correction applied every LO_EVERY-th block
W2SCALE = 64.0     # fp8 pre-scale on W2, divided out on the host

_compiled = {}
_last_in_maps = None


def _build_program(nblk):
    f32 = mybir.dt.float32
    fp8 = mybir.dt.float8e4
    Act = mybir.ActivationFunctionType
    DR = mybir.MatmulPerfMode.DoubleRow
    nsb = nblk // 2

    nc = bacc.Bacc("TRN2", target_bir_lowering=False, debug=False,
                   num_devices=N_CORES)

    in_d = nc.dram_tensor("inp", [nsb, 128, 2048], fp8, kind="ExternalInput")
    w1_d = nc.dram_tensor("w1", [4, 2, 128, 256], fp8, kind="ExternalInput")
    w2_d = nc.dram_tensor("w2", [2, 2, 128, 256], fp8, kind="ExternalInput")
    out_d = nc.dram_tensor("out", [nsb, 128, 2048], fp8, kind="ExternalOutput")

    with tile.TileContext(nc) as tc:
        with tc.tile_pool(name="consts", bufs=1) as cpool, \
             tc.tile_pool(name="work", bufs=4) as pool, \
             tc.tile_pool(name="psum", bufs=2, space="PSUM") as psum:

            w1t = {}
            for q in range(4):
                for h in range(2):
                    w = cpool.tile([128, 256], fp8, tag=f"w1_{q}{h}")
                    nc.sync.dma_start(w[:], w1_d[q, h])
                    w1t[q, h] = w[:].rearrange("p (k m) -> p k m", k=2)
            w2t = {}
            for h in range(2):
                for t in range(2):
                    w = cpool.tile([128, 256], fp8, tag=f"w2_{h}{t}")
                    nc.sync.dma_start(w[:], w2_d[h, t])
                    w2t[h, t] = w[:].rearrange("p (k m) -> p k m", k=2)

            for sb in range(nsb):
                it = pool.tile([128, 2048], fp8, tag="in")
                nc.sync.dma_start(it[:], in_d[sb])
                ob = pool.tile([128, 2048], fp8, tag="ob")
                for hb in range(2):
                    blk = 2 * sb + hb
                    qc = blk % 4
                    xv = it[:, hb * 1024:(hb + 1) * 1024].rearrange(
                        "p (k c) -> p k c", k=2)

                    ps1 = psum.tile([128, 1024], f32, tag="ps1")
                    nc.tensor.matmul(ps1[:, 0:512], w1t[qc, 0], xv,
                                     start=True, stop=True, perf_mode=DR)
                    nc.tensor.matmul(ps1[:, 512:1024], w1t[qc, 1], xv,
                                     start=True, stop=True, perf_mode=DR)

                    h1 = pool.tile([128, 1024], fp8, tag="h1")
                    nc.scalar.activation(h1[:], ps1[:], Act.Relu)
                    h1v = h1[:].rearrange("p (k c) -> p k c", k=2)

                    lo = blk % LO_EVERY == 0
                    ps2 = psum.tile([128, 1024], f32, tag="ps2")
                    nc.tensor.matmul(ps2[:, 0:512], w2t[0, 0], h1v,
                                     start=True, stop=not lo, perf_mode=DR)
                    if lo:
                        nc.tensor.matmul(ps2[:, 0:512], w2t[0, 1], h1v,
                                         start=False, stop=True, perf_mode=DR)
                    nc.tensor.matmul(ps2[:, 512:1024], w2t[1, 0], h1v,
                                     start=True, stop=not lo, perf_mode=DR)
                    if lo:
                        nc.tensor.matmul(ps2[:, 512:1024], w2t[1, 1], h1v,
                                         start=False, stop=True, perf_mode=DR)

                    nc.vector.tensor_copy(
                        ob[:, hb * 1024:(hb + 1) * 1024], ps2[:])

                nc.gpsimd.dma_start(out_d[sb], ob[:])

    nc.compile()
    return nc


def _get_program(nblk):
    if nblk not in _compiled:
        _compiled[nblk] = _build_program(nblk)
    return _compiled[nblk]


def _q(x):
    return np.asarray(x, np.float32).astype(FP8)


def kernel(X, Z, W1, b1, W2, b2, W3, b3, cell_to_batch, sample_idx_batch):
    X = np.asarray(X, dtype=np.float32)
    Z = np.asarray(Z, dtype=np.float32)
    W1 = np.asarray(W1, dtype=np.float32)
    b1 = np.asarray(b1, dtype=np.float32)
    W2 = np.asarray(W2, dtype=np.float32)
    b2 = np.asarray(b2, dtype=np.float32)
    W3 = np.asarray(W3, dtype=np.float32)
    b3 = np.asarray(b3, dtype=np.float32)
    c2b = np.asarray(cell_to_batch).astype(np.int64)
    sib = np.asarray(sample_idx_batch).astype(np.int64)

    n = X.shape[0]
    nseg = sib.shape[0]
    seg = sib[c2b]
    d_out = W3.shape[1]

    per_core = -(-n // N_CORES)                     # cells per core (unpadded)
    nblk = max(4, 4 * (-(-per_core // (4 * BLK))))  # multiple-of-4 blocks so
    # per-core slices stay aligned with the global 4-block q-class rotation
    ncap = nblk * BLK                               # padded cells per core
    nsb = nblk // 2

    # ---- quantized weights -------------------------------------------------
    W1x = W1[0:DX]
    W1z = W1[DX:DX + DZ]
    A1 = 8.0 * W1x
    W1x_hi = _q(A1)
    D = A1 - W1x_hi.astype(np.float32)
    W1x_corr16 = _q(16.0 * D)
    W1z_hi = _q(W1z)
    W1z_r = _q(W1z / 2.0)
    W1z_lo16 = _q(16.0 * (W1z - W1z_hi.astype(np.float32)))
    W1z_lo64 = _q(64.0 * (W1z - W1z_hi.astype(np.float32)))
    b1_hi = _q(b1)
    b1_lo16 = _q(16.0 * (b1 - b1_hi.astype(np.float32)))

    # w1 stationary tiles [q, half, p, k*128+m]
    w1q = np.zeros((4, 2, 128, 256), dtype=FP8)
    zr_base = np.zeros((128, H), dtype=FP8)
    zr_base[0:32] = W1z_hi
    zr_base[32:64] = W1z_r
    zr_base[64] = b1_hi
    zr_base[65] = b1_lo16
    zr_base[66:96] = W1z_lo16[0:30]
    for q in range(4):
        zr = zr_base.copy()
        if q < 3:
            zr[96:128] = W1x_corr16[32 * q:32 * q + 32]
        else:
            zr[96:126] = W1x_corr16[96:126]
            zr[126:128] = W1z_lo64[30:32]
        for h in range(2):
            w1q[q, h, :, 0:128] = W1x_hi[:, h * 128:(h + 1) * 128]
            w1q[q, h, :, 128:256] = zr[:, h * 128:(h + 1) * 128]

    # w2 stationary tiles [half, term, p, k*128+m] (k = h1 row-block)
    A2 = W2SCALE * W2
    W2_hi = _q(A2)
    W2_lo = _q(LO_EVERY * (A2 - W2_hi.astype(np.float32)))
    w2q = np.zeros((2, 2, 128, 256), dtype=FP8)
    for t, term in enumerate((W2_hi, W2_lo)):
        km = term.reshape(2, 128, H).transpose(1, 0, 2)  # [p, k, m]
        for h in range(2):
            w2q[h, t] = km[:, :, h * 128:(h + 1) * 128].reshape(128, 256)

    # ---- per-cell quantized activations ------------------------------------
    lx = np.log1p(X)
    Xs = _q(lx / 8.0)                  # [n, 128]
    Xs32 = _q(lx / 32.0)
    Z_hi = _q(Z)
    Z_r = _q(2.0 * (Z - Z_hi.astype(np.float32)))
    Z_hi16 = _q(Z_hi.astype(np.float32) / 16.0)
    Zc_hi = Z_hi[c2b]
    Zc_r = Z_r[c2b]
    Zc_hi16 = Z_hi16[c2b]

    zp = np.zeros((n, 128), dtype=FP8)
    zp[:, 0:32] = Zc_hi
    zp[:, 32:64] = Zc_r
    zp[:, 64] = FP8(1.0)
    zp[:, 65] = FP8(0.0625)
    zp[:, 66:96] = Zc_hi16[:, 0:30]
    qcls = (np.arange(n) // BLK) % 4   # block class before core split
    # rows 96-127: rotating X-weight corrections (q<3) / leftovers (q==3)
    for q in range(3):
        m = qcls == q
        zp[m, 96:128] = Xs32[m][:, 32 * q:32 * q + 32]
    m = qcls == 3
    zp[m, 96:126] = Xs32[m][:, 96:126]
    zp[m, 126:128] = Zc_hi16[m][:, 30:32]

    # NOTE: block classes are computed on the GLOBAL cell index, and cores get
    # contiguous slices of ncap cells, so per-core block boundaries align with
    # the global ones only if per-core offsets are multiples of 4*BLK. ncap is
    # nblk*BLK with nblk even; ensure nblk % 4 == 0 so q-classes stay aligned.

    # ---- assemble per-core input blocks ------------------------------------
    in_arr = np.zeros((N_CORES, nsb, 128, 2048), dtype=FP8)
    for c in range(N_CORES):
        s, e = c * ncap, min(n, (c + 1) * ncap)
        if s >= e:
            continue
        cnt = e - s
        xs_c = np.zeros((ncap, 128), dtype=FP8)
        xs_c[:cnt] = Xs[s:e]
        zp_c = np.zeros((ncap, 128), dtype=FP8)
        zp_c[:cnt] = zp[s:e]
        xs_r = xs_c.reshape(nsb, 2, BLK, 128)
        zp_r = zp_c.reshape(nsb, 2, BLK, 128)
        for hb in range(2):
            in_arr[c, :, :, hb * 1024:hb * 1024 + 512] = \
                xs_r[:, hb].transpose(0, 2, 1)
            in_arr[c, :, :, hb * 1024 + 512:(hb + 1) * 1024] = \
                zp_r[:, hb].transpose(0, 2, 1)

    # ---- run on 8 cores ----------------------------------------------------
    nc = _get_program(nblk)
    in_maps = []
    for c in range(N_CORES):
        in_maps.append({"inp": in_arr[c], "w1": w1q, "w2": w2q})
    global _last_in_maps
    _last_in_maps = in_maps
    res = run_bass_kernel_spmd(nc, in_maps, list(range(N_CORES)))

    # ---- host epilogue -----------------------------------------------------
    sums = np.zeros((nseg, d_out), np.float32)
    lut = np.arange(256, dtype=np.uint8).view(FP8).astype(np.float32)
    for c in range(N_CORES):
        s, e = c * ncap, min(n, (c + 1) * ncap)
        if s >= e:
            continue
        cnt = e - s
        o = res.results[c]["out"]                      # [nsb, 128, 2048] fp8
        o = lut[o.view(np.uint8)]
        o = o.reshape(nsb, 128, 2, 2, BLK)             # [sb, p, hb, half, c]
        o = o.transpose(0, 2, 4, 3, 1).reshape(ncap, H)[:cnt]
        h2 = np.maximum(o / W2SCALE + b2[None, :], 0.0)
        y16 = h2 @ W3
        seg_c = seg[s:e]
        for j in range(d_out):
            sums[:, j] += np.bincount(seg_c, weights=y16[:, j],
                                      minlength=nseg)

    counts = np.bincount(seg, minlength=nseg).astype(np.float32)
    Y = sums / np.maximum(counts, 1.0)[:, None] + b3[None, :]
    Y[counts == 0] = 0.0
    return Y.astype(np.float32)


# revision 6
# speedup vs baseline: 2.0767x; 1.0144x over previous
"""Trainium2 Bass kernel for CompositionModel (gnn_message_passing), v2.

Model: per-cell MLP over [log1p(X) ++ Z[cell_to_batch]] followed by a
segment-mean over batch labels.

Strategy (all-fp8 device pipeline, host reduce):
  * Host: precompute log1p(X) and quantize everything to fp8 e4m3.
    Per 512-cell block the device sees one [128, 1024] fp8 tile:
    cols 0-511 = Xs = fp8(log1p(X)/8) (features on partitions), cols
    512-1023 a "Z-pack" k-tile whose rows carry Zc_hi, Zc_residual,
    constant rows for an exact (hi + lo/16) b1, Z-weight-correction
    rows, and a 4-block rotation of X-weight-correction rows. With
    weights quantized as hi + scaled-lo pairs, layer 1 is a SINGLE
    DoubleRow matmul per 128-wide output half - corrections included.
  * Layer 2 = fp8 DoubleRow vs W2SCALE-scaled fp8 W2 (hi every block,
    pre-scaled lo every LO_EVERY-th block, first-order exact through
    the relu + segment mean).
  * Device per block: 2 DR matmuls -> ACT relu (fp8 h1) -> 2(+2) DR
    matmuls -> DVE cast (raw fp8 ps2) -> SWDGE store. No bias, no
    relu2, no reduction on device.
  * Host epilogue: decode fp8 ps2, add b2, relu, apply W3/b3 and the
    segment mean in f32 numpy.
"""

import numpy as np
import ml_dtypes

import concourse.bacc as bacc
import concourse.mybir as mybir
import concourse.tile as tile
from concourse.bass_utils import run_bass_kernel_spmd

BF16 = ml_dtypes.bfloat16
FP8 = ml_dtypes.float8_e4m3fn

N_CORES = 8
DX = 128
DZ = 32
H = 256
BLK = 512          # cells per block
NBLK = 124         # blocks per core for the 500k-cell reference input
LO_EVERY = 2       # W2 lo-correction applied every LO_EVERY-th block
W2SCALE = 64.0     # fp8 pre-scale on W2, divided out on the host

_compiled = {}
_last_in_maps = None


def _build_program(nblk):
    f32 = mybir.dt.float32
    fp8 = mybir.dt.float8e4
    Act = mybir.ActivationFunctionType
    DR = mybir.MatmulPerfMode.DoubleRow
    nsb = nblk // 2

    nc = bacc.Bacc("TRN2", target_bir_lowering=False, debug=False,
                   num_devices=N_CORES)

    in_d = nc.dram_tensor("inp", [nsb, 128, 2048], fp8, kind="ExternalInput")
    w1_d = nc.dram_tensor("w1", [4, 2, 128, 256], fp8, kind="ExternalInput")
    w2_d = nc.dram_tensor("w2", [2, 2, 128, 256], fp8, kind="ExternalInput")
    out_d = nc.dram_tensor("out", [nsb, 128, 2048], fp8, kind="ExternalOutput")

    with tile.TileContext(nc) as tc:
        with tc.tile_pool(name="consts", bufs=1) as cpool, \
             tc.tile_pool(name="work", bufs=6) as pool, \
             tc.tile_pool(name="psum", bufs=2, space="PSUM") as psum:

            # weight preamble on the scalar HWDGE queue so it overlaps the
            # first input-tile loads on the sync queue
            w1t = {}
            for q in range(4):
                for h in range(2):
                    w = cpool.tile([128, 256], fp8, tag=f"w1_{q}{h}")
                    nc.scalar.dma_start(w[:], w1_d[q, h])
                    w1t[q, h] = w[:].rearrange("p (k m) -> p k m", k=2)
            w2t = {}
            for h in range(2):
                for t in range(2):
                    w = cpool.tile([128, 256], fp8, tag=f"w2_{h}{t}")
                    nc.scalar.dma_start(w[:], w2_d[h, t])
                    w2t[h, t] = w[:].rearrange("p (k m) -> p k m", k=2)

            # Software-pipelined over superblocks with a one-iteration skew:
            # iteration p runs L1(+relu) for pair p while L2(+cast+store) run
            # for pair p-1, so the in-order PE queue never waits on the relu
            # of the same block between its L1 and L2 matmul groups.
            def emit_l1(p):
                it = pool.tile([128, 2048], fp8, tag="in")
                nc.sync.dma_start(it[:], in_d[p])
                out = []
                for hb in range(2):
                    blk = 2 * p + hb
                    qc = blk % 4
                    xv = it[:, hb * 1024:(hb + 1) * 1024].rearrange(
                        "p (k c) -> p k c", k=2)
                    ps1 = psum.tile([128, 1024], f32, tag="ps1")
                    nc.tensor.matmul(ps1[:, 0:512], w1t[qc, 0], xv,
                                     start=True, stop=True, perf_mode=DR)
                    nc.tensor.matmul(ps1[:, 512:1024], w1t[qc, 1], xv,
                                     start=True, stop=True, perf_mode=DR)
                    out.append(ps1)
                return out

            def emit_relu(ps1_pair):
                out = []
                for ps1 in ps1_pair:
                    h1 = pool.tile([128, 1024], fp8, tag="h1")
                    nc.scalar.activation(h1[:], ps1[:], Act.Relu)
                    out.append(h1)
                return out

            def emit_l2(p, h1_pair):
                out = []
                for hb in range(2):
                    blk = 2 * p + hb
                    h1v = h1_pair[hb][:].rearrange("p (k c) -> p k c", k=2)
                    lo = blk % LO_EVERY == 0
                    ps2 = psum.tile([128, 1024], f32, tag="ps2")
                    nc.tensor.matmul(ps2[:, 0:512], w2t[0, 0], h1v,
                                     start=True, stop=not lo, perf_mode=DR)
                    if lo:
                        nc.tensor.matmul(ps2[:, 0:512], w2t[0, 1], h1v,
                                         start=False, stop=True, perf_mode=DR)
                    nc.tensor.matmul(ps2[:, 512:1024], w2t[1, 0], h1v,
                                     start=True, stop=not lo, perf_mode=DR)
                    if lo:
                        nc.tensor.matmul(ps2[:, 512:1024], w2t[1, 1], h1v,
                                         start=False, stop=True, perf_mode=DR)
                    out.append(ps2)
                return out

            def emit_cast_store(p, ps2_pair):
                ob = pool.tile([128, 2048], fp8, tag="ob")
                for hb in range(2):
                    nc.vector.tensor_copy(
                        ob[:, hb * 1024:(hb + 1) * 1024], ps2_pair[hb][:])
                nc.gpsimd.dma_start(out_d[p], ob[:])

            h1s = {}
            for p in range(nsb + 2):
                if p >= 2:
                    ps2_pair = emit_l2(p - 2, h1s.pop(p - 2))
                if p < nsb:
                    ps1_pair = emit_l1(p)
                if p >= 2:
                    emit_cast_store(p - 2, ps2_pair)
                if p < nsb:
                    h1s[p] = emit_relu(ps1_pair)

    nc.compile()
    return nc


def _get_program(nblk):
    if nblk not in _compiled:
        _compiled[nblk] = _build_program(nblk)
    return _compiled[nblk]


def _q(x):
    return np.asarray(x, np.float32).astype(FP8)


def kernel(X, Z, W1, b1, W2, b2, W3, b3, cell_to_batch, sample_idx_batch):
    X = np.asarray(X, dtype=np.float32)
    Z = np.asarray(Z, dtype=np.float32)
    W1 = np.asarray(W1, dtype=np.float32)
    b1 = np.asarray(b1, dtype=np.float32)
    W2 = np.asarray(W2, dtype=np.float32)
    b2 = np.asarray(b2, dtype=np.float32)
    W3 = np.asarray(W3, dtype=np.float32)
    b3 = np.asarray(b3, dtype=np.float32)
    c2b = np.asarray(cell_to_batch).astype(np.int64)
    sib = np.asarray(sample_idx_batch).astype(np.int64)

    n = X.shape[0]
    nseg = sib.shape[0]
    seg = sib[c2b]
    d_out = W3.shape[1]

    per_core = -(-n // N_CORES)                     # cells per core (unpadded)
    nblk = max(4, 4 * (-(-per_core // (4 * BLK))))  # multiple-of-4 blocks so
    # per-core slices stay aligned with the global 4-block q-class rotation
    ncap = nblk * BLK                               # padded cells per core
    nsb = nblk // 2

    # ---- quantized weights -------------------------------------------------
    W1x = W1[0:DX]
    W1z = W1[DX:DX + DZ]
    A1 = 8.0 * W1x
    W1x_hi = _q(A1)
    D = A1 - W1x_hi.astype(np.float32)
    W1x_corr16 = _q(16.0 * D)
    W1z_hi = _q(W1z)
    W1z_r = _q(W1z / 2.0)
    W1z_lo16 = _q(16.0 * (W1z - W1z_hi.astype(np.float32)))
    W1z_lo64 = _q(64.0 * (W1z - W1z_hi.astype(np.float32)))
    b1_hi = _q(b1)
    b1_lo16 = _q(16.0 * (b1 - b1_hi.astype(np.float32)))

    # w1 stationary tiles [q, half, p, k*128+m]
    w1q = np.zeros((4, 2, 128, 256), dtype=FP8)
    zr_base = np.zeros((128, H), dtype=FP8)
    zr_base[0:32] = W1z_hi
    zr_base[32:64] = W1z_r
    zr_base[64] = b1_hi
    zr_base[65] = b1_lo16
    zr_base[66:96] = W1z_lo16[0:30]
    for q in range(4):
        zr = zr_base.copy()
        if q < 3:
            zr[96:128] = W1x_corr16[32 * q:32 * q + 32]
        else:
            zr[96:126] = W1x_corr16[96:126]
            zr[126:128] = W1z_lo64[30:32]
        for h in range(2):
            w1q[q, h, :, 0:128] = W1x_hi[:, h * 128:(h + 1) * 128]
            w1q[q, h, :, 128:256] = zr[:, h * 128:(h + 1) * 128]

    # w2 stationary tiles [half, term, p, k*128+m] (k = h1 row-block)
    A2 = W2SCALE * W2
    W2_hi = _q(A2)
    W2_lo = _q(LO_EVERY * (A2 - W2_hi.astype(np.float32)))
    w2q = np.zeros((2, 2, 128, 256), dtype=FP8)
    for t, term in enumerate((W2_hi, W2_lo)):
        km = term.reshape(2, 128, H).transpose(1, 0, 2)  # [p, k, m]
        for h in range(2):
            w2q[h, t] = km[:, :, h * 128:(h + 1) * 128].reshape(128, 256)

    # ---- per-cell quantized activations ------------------------------------
    lx = np.log1p(X)
    Xs = _q(lx / 8.0)                  # [n, 128]
    Xs32 = _q(lx / 32.0)
    Z_hi = _q(Z)
    Z_r = _q(2.0 * (Z - Z_hi.astype(np.float32)))
    Z_hi16 = _q(Z_hi.astype(np.float32) / 16.0)
    Zc_hi = Z_hi[c2b]
    Zc_r = Z_r[c2b]
    Zc_hi16 = Z_hi16[c2b]

    zp = np.zeros((n, 128), dtype=FP8)
    zp[:, 0:32] = Zc_hi
    zp[:, 32:64] = Zc_r
    zp[:, 64] = FP8(1.0)
    zp[:, 65] = FP8(0.0625)
    zp[:, 66:96] = Zc_hi16[:, 0:30]
    qcls = (np.arange(n) // BLK) % 4   # block class before core split
    # rows 96-127: rotating X-weight corrections (q<3) / leftovers (q==3)
    for q in range(3):
        m = qcls == q
        zp[m, 96:128] = Xs32[m][:, 32 * q:32 * q + 32]
    m = qcls == 3
    zp[m, 96:126] = Xs32[m][:, 96:126]
    zp[m, 126:128] = Zc_hi16[m][:, 30:32]

    # NOTE: block classes are computed on the GLOBAL cell index, and cores get
    # contiguous slices of ncap cells, so per-core block boundaries align with
    # the global ones only if per-core offsets are multiples of 4*BLK. ncap is
    # nblk*BLK with nblk even; ensure nblk % 4 == 0 so q-classes stay aligned.

    # ---- assemble per-core input blocks ------------------------------------
    in_arr = np.zeros((N_CORES, nsb, 128, 2048), dtype=FP8)
    for c in range(N_CORES):
        s, e = c * ncap, min(n, (c + 1) * ncap)
        if s >= e:
            continue
        cnt = e - s
        xs_c = np.zeros((ncap, 128), dtype=FP8)
        xs_c[:cnt] = Xs[s:e]
        zp_c = np.zeros((ncap, 128), dtype=FP8)
        zp_c[:cnt] = zp[s:e]
        xs_r = xs_c.reshape(nsb, 2, BLK, 128)
        zp_r = zp_c.reshape(nsb, 2, BLK, 128)
        for hb in range(2):
            in_arr[c, :, :, hb * 1024:hb * 1024 + 512] = \
                xs_r[:, hb].transpose(0, 2, 1)
            in_arr[c, :, :, hb * 1024 + 512:(hb + 1) * 1024] = \
                zp_r[:, hb].transpose(0, 2, 1)

    # ---- run on 8 cores ----------------------------------------------------
    nc = _get_program(nblk)
    in_maps = []
    for c in range(N_CORES):
        in_maps.append({"inp": in_arr[c], "w1": w1q, "w2": w2q})
    global _last_in_maps
    _last_in_maps = in_maps
    res = run_bass_kernel_spmd(nc, in_maps, list(range(N_CORES)))

    # ---- host epilogue -----------------------------------------------------
    sums = np.zeros((nseg, d_out), np.float32)
    lut = np.arange(256, dtype=np.uint8).view(FP8).astype(np.float32)
    for c in range(N_CORES):
        s, e = c * ncap, min(n, (c + 1) * ncap)
        if s >= e:
            continue
        cnt = e - s
        o = res.results[c]["out"]                      # [nsb, 128, 2048] fp8
        o = lut[o.view(np.uint8)]
        o = o.reshape(nsb, 128, 2, 2, BLK)             # [sb, p, hb, half, c]
        o = o.transpose(0, 2, 4, 3, 1).reshape(ncap, H)[:cnt]
        h2 = np.maximum(o / W2SCALE + b2[None, :], 0.0)
        y16 = h2 @ W3
        seg_c = seg[s:e]
        for j in range(d_out):
            sums[:, j] += np.bincount(seg_c, weights=y16[:, j],
                                      minlength=nseg)

    counts = np.bincount(seg, minlength=nseg).astype(np.float32)
    Y = sums / np.maximum(counts, 1.0)[:, None] + b3[None, :]
    Y[counts == 0] = 0.0
    return Y.astype(np.float32)
